# revision 1
# baseline (speedup 1.0000x reference)
"""LocalityEnhancedAttention Trainium2 kernel (8 NeuronCores, SPMD).

Sharding: core c handles batch b = c//2 and head-group g = c%2
(8 of 16 heads). Each core computes its partial output projection
(attn_heads @ wo_shard.T); host sums the two head-group partials per
batch and adds bo.

Device dataflow per core (S=2048, M=1024, local head-dims DH=512):
  - host pre-transposes inputs:  qT/kT/vT = x[b].T  [M, S]
  - projections (f32r matmuls): Q^T,K^T [DH, S] in [d, s] layout,
    V in [s, d] layout augmented with a ones column per head (rowsums)
  - scores^T[kj, qi] = K^T.T @ Q^T per head, head pairs packed into
    PE row-halves (contraction = Dk = 64), banded local bias added via
    DVE, exp via ACT (scale=1/8 folded in), P^T in SBUF
  - PV: A^T_aug[65, qi] += V_aug[kj].T @ P^T[kj] accumulated in PSUM;
    row 64 = softmax denominators.  Normalize via DVE recip +
    gpsimd partition_broadcast + DVE mul.
  - WO: out_partial[s, m] = sum_d A_norm^T.T @ woT
"""

import os
import sys
from contextlib import ExitStack

import numpy as np

sys.path.insert(0, "/opt/trn_rl_repo")

import ml_dtypes

BF = ml_dtypes.bfloat16

import concourse.bass as bass
import concourse.mybir as mybir
import concourse.tile as tile
from concourse import bacc
from concourse.bass_utils import run_bass_kernel_spmd

F32 = mybir.dt.float32
F32R = mybir.dt.float32r
BF16 = mybir.dt.bfloat16
EXP = mybir.ActivationFunctionType.Exp
TS, DS = bass.ts, bass.ds

S = 2048
M = 1024
DH = 512        # head dims per core (8 heads x 64)
DK = 64
W = 16
NPT = 4         # head pairs per core
NCH = 4         # qi chunks of 512
NKJ = 16        # kj tiles of 128


def _emit(ctx, tc, io):
    nc = tc.nc

    const = ctx.enter_context(tc.tile_pool(name="const", bufs=1))
    qkvp = ctx.enter_context(tc.tile_pool(name="qkv", bufs=1))
    ap_ = ctx.enter_context(tc.tile_pool(name="anorm", bufs=1))
    wop = ctx.enter_context(tc.tile_pool(name="wop", bufs=1))

    pat = const.tile([128, 320], BF16, tag="pat", name="pat")
    nc.sync.dma_start(pat[:], io["pat"])
    ones_r = const.tile([1, 512], BF16, tag="ones_r", name="ones_r")
    nc.vector.memset(ones_r[:], 1.0)
    bq = const.tile([1, DH], BF16, tag="bq", name="bq")
    nc.sync.dma_start(bq[:], io["bq"])
    bk = const.tile([1, DH], BF16, tag="bk", name="bk")
    nc.sync.dma_start(bk[:], io["bk"])
    bv = const.tile([1, DH], BF16, tag="bv", name="bv")
    nc.sync.dma_start(bv[:], io["bv"])

    qT_sb = [qkvp.tile([128, S], BF16, tag=f"q{i}", name=f"q{i}") for i in range(NPT)]
    kT_sb = [qkvp.tile([128, S], BF16, tag=f"k{i}", name=f"k{i}") for i in range(NPT)]
    v_sb = [qkvp.tile([128, 8 * 65], BF16, tag=f"v{i}", name=f"v{i}") for i in range(16)]
    a_sb = [ap_.tile([128, S], BF16, tag=f"a{i}", name=f"a{i}") for i in range(NPT)]
    woT_sb = [wop.tile([128, M], BF16, tag=f"wo{i}", name=f"wo{i}") for i in range(NPT)]
    for i in range(NPT):
        nc.sync.dma_start(woT_sb[i][:], io["woT"][TS(i, 128), :])

    # ---------------- projections ----------------
    with ExitStack() as ps:
        wpool = ps.enter_context(tc.tile_pool(name="wpool", bufs=10))
        stream = ps.enter_context(tc.tile_pool(name="stream", bufs=12))
        pproj = ps.enter_context(tc.tile_pool(name="pproj", bufs=2, space="PSUM"))

        # V projection -> v_sb[st] = [128, 8*65] ([s, head-major d | ones])
        wv = []
        for k in range(8):
            t = wpool.tile([128, DH], BF16, tag="w", name="w")
            nc.sync.dma_start(t[:], io["wvT"][TS(k, 128), :])
            wv.append(t)
        for st in range(16):
            vv = v_sb[st].rearrange("p (h e) -> p h e", e=65)
            nc.vector.memset(vv[:, :, 64:65], 1.0)
        for sc in range(4):  # s-chunks of 512
            vs = []
            for k in range(8):
                t = stream.tile([128, 512], BF16, tag="s", name="s")
                nc.sync.dma_start(t[:], io["vT"][TS(k, 128), TS(sc, 512)])
                vs.append(t)
            for j in range(4):
                st = sc * 4 + j
                psv = pproj.tile([128, DH], F32, tag="pp", name="pp")
                for k in range(8):
                    nc.tensor.matmul(
                        psv[:],
                        lhsT=vs[k][:, TS(j, 128)],
                        rhs=wv[k][:],
                        start=(k == 0), stop=False, skip_group_check=True)
                nc.tensor.matmul(
                    psv[:],
                    lhsT=ones_r[0:1, 0:128],
                    rhs=bv[:],
                    start=False, stop=True, skip_group_check=True)
                vv = v_sb[st].rearrange("p (h e) -> p h e", e=65)
                pv_view = psv.rearrange("p (h e) -> p h e", e=64)
                nc.vector.tensor_copy(vv[:, :, 0:64], pv_view[:, :, :])

        # Q^T and K^T projections -> [d, s] layout pair tiles
        for (w_name, x_name, b_tile, dst) in (
            ("wqT", "qT", bq, qT_sb),
            ("wkT", "kT", bk, kT_sb),
        ):
            wt = []
            for k in range(8):
                t = wpool.tile([128, DH], BF16, tag="w", name="w")
                nc.sync.dma_start(t[:], io[w_name][TS(k, 128), :])
                wt.append(t)
            for sc in range(4):
                xs = []
                for k in range(8):
                    t = stream.tile([128, 512], BF16, tag="s", name="s")
                    nc.sync.dma_start(t[:], io[x_name][TS(k, 128), TS(sc, 512)])
                    xs.append(t)
                for pt in range(NPT):
                    psq = pproj.tile([128, 512], F32, tag="pp", name="pp")
                    for k in range(8):
                        nc.tensor.matmul(
                            psq[:],
                            lhsT=wt[k][:, TS(pt, 128)],
                            rhs=xs[k][:],
                            start=(k == 0), stop=False, skip_group_check=True)
                    nc.tensor.matmul(
                        psq[:],
                        lhsT=b_tile[0:1, TS(pt, 128)],
                        rhs=ones_r[:],
                        start=False, stop=True, skip_group_check=True)
                    nc.vector.tensor_copy(dst[pt][:, TS(sc, 512)], psq[:])

    # ---------------- attention + output projection ----------------
    with ExitStack() as asx:
        ptp = asx.enter_context(tc.tile_pool(name="ptp", bufs=4))
        rp = asx.enter_context(tc.tile_pool(name="rp", bufs=6))
        op = asx.enter_context(tc.tile_pool(name="op", bufs=4))
        sps = asx.enter_context(tc.tile_pool(name="sps", bufs=2, space="PSUM"))
        pvs = asx.enter_context(tc.tile_pool(name="pvs", bufs=2, space="PSUM"))
        wops = asx.enter_context(tc.tile_pool(name="wops", bufs=2, space="PSUM"))

        patv = pat.rearrange("p (h w) -> p h w", h=2)
        for ch in range(NCH):
            c0 = ch * 512
            for pt in range(NPT):
                pvt = [pvs.tile([65, 512], F32, tag="pv", name="pv") for _ in range(2)]
                for kj in range(16):
                    kj0 = kj * 128
                    sp = sps.tile([128, 1024], F32, tag="sp", name="sp")
                    for h in (0, 1):
                        nc.tensor.matmul(
                            sp[:, DS(h * 512, 512)],
                            lhsT=kT_sb[pt][DS(h * 64, 64), TS(kj, 128)],
                            rhs=qT_sb[pt][DS(h * 64, 64), TS(ch, 512)],
                            start=True, stop=True,
                            tile_position=(h * 64, 0),
                            skip_group_check=True)
                    ptt = ptp.tile([128, 1024], BF16, tag="ptt", name="ptt")
                    nc.scalar.activation(ptt[:], sp[:], EXP, scale=0.125)
                    lo = max(kj0 - W, c0)
                    hi = min(kj0 + 128 + W, c0 + 512)
                    if lo < hi:
                        pa = lo - (kj0 - W)
                        pv3 = ptt.rearrange("p (h w) -> p h w", h=2)
                        nc.vector.tensor_mul(
                            pv3[:, :, DS(lo - c0, hi - lo)],
                            pv3[:, :, DS(lo - c0, hi - lo)],
                            patv[:, :, DS(pa, hi - lo)])
                    for h in (0, 1):
                        lh = pt * 2 + h
                        nc.tensor.matmul(
                            pvt[h][:],
                            lhsT=v_sb[kj][:, DS(lh * 65, 65)],
                            rhs=ptt[:, DS(h * 512, 512)],
                            start=(kj == 0), stop=(kj == 15),
                            skip_group_check=True)
                for h in (0, 1):
                    r1 = rp.tile([1, 512], F32, tag="r1", name="r1")
                    nc.vector.reciprocal(r1[:], pvt[h][DS(64, 1), :])
                    rb = rp.tile([64, 512], F32, tag="rb", name="rb")
                    nc.gpsimd.partition_broadcast(rb[:], r1[:])
                    nc.vector.tensor_mul(
                        a_sb[pt][DS(h * 64, 64), TS(ch, 512)],
                        pvt[h][DS(0, 64), :], rb[:])
            for j in range(4):
                st = ch * 4 + j
                for mt in range(2):
                    pso = wops.tile([128, 512], F32, tag="pso", name="pso")
                    for pt in range(NPT):
                        nc.tensor.matmul(
                            pso[:],
                            lhsT=a_sb[pt][:, TS(st, 128)],
                            rhs=woT_sb[pt][:, TS(mt, 512)],
                            start=(pt == 0), stop=(pt == 3),
                            skip_group_check=True)
                    ot = op.tile([128, 512], F32, tag="ot", name="ot")
                    nc.vector.tensor_copy(ot[:], pso[:])
                    nc.sync.dma_start(io["out"][TS(st, 128), TS(mt, 512)], ot[:])


_CACHE = {}


def _build():
    if "nc" in _CACHE:
        return _CACHE["nc"]
    nc = bacc.Bacc("TRN2", target_bir_lowering=False, debug=False)
    io = {}
    for name, shape in (
        ("qT", [M, S]), ("kT", [M, S]), ("vT", [M, S]),
        ("wqT", [M, DH]), ("wkT", [M, DH]), ("wvT", [M, DH]),
        ("woT", [DH, M]),
        ("bq", [1, DH]), ("bk", [1, DH]), ("bv", [1, DH]),    ):
        io[name] = nc.dram_tensor(name, shape, BF16, kind="ExternalInput").ap()
    io["pat"] = nc.dram_tensor("pat", [128, 320], BF16, kind="ExternalInput").ap()
    io["out"] = nc.dram_tensor("out", [S, M], F32, kind="ExternalOutput").ap()
    with tile.TileContext(nc) as tc:
        with ExitStack() as ctx:
            _emit(ctx, tc, io)
    nc.compile()
    _CACHE["nc"] = nc
    return nc


def _bias_pattern(local_bias):
    # multiplicative band pattern: exp(2*b[qi-kj+W]) inside the band, 1.0
    # outside; duplicated side by side for the two heads of a pair tile.
    p = np.arange(128)[:, None]
    f = np.arange(160)[None, :]
    idx = f - p  # rel + W
    valid = (idx >= 0) & (idx <= 2 * W)
    b = np.asarray(local_bias, np.float64)
    pat = np.where(valid, np.exp(2.0 * b[np.clip(idx, 0, 2 * W)]), 1.0)
    pat2 = np.concatenate([pat, pat], axis=1)
    return np.ascontiguousarray(pat2).astype(BF)


def kernel(query, key, value, wq, bq, wk, bk, wv, bv, wo, bo, local_bias):
    query = np.asarray(query, np.float32)
    key = np.asarray(key, np.float32)
    value = np.asarray(value, np.float32)
    wq, wk, wv, wo = (np.asarray(x, np.float32) for x in (wq, wk, wv, wo))
    bq, bk, bv, bo = (np.asarray(x, np.float32) for x in (bq, bk, bv, bo))
    pat = _bias_pattern(local_bias)

    nc = _build()
    in_maps = []
    for c in range(8):
        b, g = c // 2, c % 2
        sl = slice(g * DH, (g + 1) * DH)
        in_maps.append({
            "qT": np.ascontiguousarray(query[b].T).astype(BF),
            "kT": np.ascontiguousarray(key[b].T).astype(BF),
            "vT": np.ascontiguousarray(value[b].T).astype(BF),
            "wqT": np.ascontiguousarray(wq[sl, :].T).astype(BF),
            "wkT": np.ascontiguousarray(wk[sl, :].T).astype(BF),
            "wvT": np.ascontiguousarray(wv[sl, :].T).astype(BF),
            "woT": np.ascontiguousarray(wo[:, sl].T).astype(BF),
            "bq": np.ascontiguousarray(bq[sl]).reshape(1, DH).astype(BF),
            "bk": np.ascontiguousarray(bk[sl]).reshape(1, DH).astype(BF),
            "bv": np.ascontiguousarray(bv[sl]).reshape(1, DH).astype(BF),
            "pat": pat,
        })
    res = run_bass_kernel_spmd(
        nc, in_maps, core_ids=list(range(8)),
        trace=bool(int(os.environ.get("KERNEL_TRACE", "0"))),
    )
    _CACHE["last_result"] = res
    outs = [r["out"] for r in res.results]
    out = np.stack([outs[2 * b] + outs[2 * b + 1] + bo for b in range(4)])
    return out.astype(np.float32)



# revision 3
# speedup vs baseline: 1.0604x; 1.0604x over previous
"""LocalityEnhancedAttention Trainium2 kernel (8 NeuronCores, SPMD).

Sharding: core c handles batch b = c//2 and head-group g = c%2
(8 of 16 heads). Each core computes its partial output projection
(attn_heads @ wo_shard.T); host sums the two head-group partials per
batch and adds bo + wo@bv (the V-bias folds out of the device entirely
because softmax rows sum to one; the K-bias shifts every logit in a row
equally so it cancels in softmax; the Q-bias is added during the Q
projection's PSUM->SBUF copy as a per-partition ACT bias).

Device dataflow per core (S=2048, M=1024, local head-dims DH=512):
  - host pre-transposes inputs:  qT/kT/vT = x[b].T  [M, S]
  - projections: K^T, V, then Q^T chunk-by-chunk; attention for query
    chunk 0 is emitted right after Q chunk 0 so the Q tail overlaps.
  - scores^T[kj, qi] = K^T.T @ Q^T per head, head pairs packed into
    PE row-halves, exp via ACT (scale=1/8 folded), banded local bias
    applied multiplicatively on the bf16 probs (DVE), P^T in SBUF.
  - PV: A^T_aug[65, qi] += V_aug[kj].T @ P^T[kj] in PSUM; row 64 =
    softmax denominators; PV lags scores by one kj tile so the tensor
    queue never blocks on the exp of the tile it just produced.
  - normalize: reciprocal_approx_fast + gpsimd partition_broadcast +
    DVE mul -> A_norm^T in SBUF (bf16).
  - WO: out_partial[s, m] = sum_d A_norm^T.T @ woT  (bf16 out).
"""

import os
import sys
from contextlib import ExitStack

import numpy as np

sys.path.insert(0, "/opt/trn_rl_repo")

import ml_dtypes

BF = ml_dtypes.bfloat16

import concourse.bass as bass
import concourse.mybir as mybir
import concourse.tile as tile
from concourse import bacc
from concourse.bass_utils import run_bass_kernel_spmd

F32 = mybir.dt.float32
BF16 = mybir.dt.bfloat16
EXP = mybir.ActivationFunctionType.Exp
IDENT = mybir.ActivationFunctionType.Identity
TS, DS = bass.ts, bass.ds

S = 2048
M = 1024
DH = 512        # head dims per core (8 heads x 64)
DK = 64
W = 16
NPT = 4         # head pairs per core
NCH = 4         # qi chunks of 512
NKJ = 16        # kj tiles of 128


def _emit(ctx, tc, io):
    nc = tc.nc

    const = ctx.enter_context(tc.tile_pool(name="const", bufs=1))
    qkvp = ctx.enter_context(tc.tile_pool(name="qkv", bufs=1))
    ap_ = ctx.enter_context(tc.tile_pool(name="anorm", bufs=1))
    wop = ctx.enter_context(tc.tile_pool(name="wop", bufs=1))

    pat = const.tile([128, 320], BF16, tag="pat", name="pat")
    nc.sync.dma_start(pat[:], io["pat"])
    bqT = const.tile([128, NPT], F32, tag="bqT", name="bqT")
    nc.sync.dma_start(bqT[:], io["bqT"])

    qT_sb = [qkvp.tile([128, S], BF16, tag=f"q{i}", name=f"q{i}") for i in range(NPT)]
    kT_sb = [qkvp.tile([128, S], BF16, tag=f"k{i}", name=f"k{i}") for i in range(NPT)]
    v_sb = [qkvp.tile([128, 8 * 65], BF16, tag=f"v{i}", name=f"v{i}") for i in range(16)]
    a_sb = [ap_.tile([128, S], BF16, tag=f"a{i}", name=f"a{i}") for i in range(NPT)]
    woT_sb = wop.tile([128, NPT, M], BF16, tag="wo", name="wo")
    nc.sync.dma_start(
        woT_sb[:], io["woT"].rearrange("(t p) m -> p t m", t=NPT))

    # shared PSUM pool: projection accumulators AND the WO accumulators
    pp = ctx.enter_context(tc.tile_pool(name="pp", bufs=2, space="PSUM"))
    # attention PSUM pools
    sps = ctx.enter_context(tc.tile_pool(name="sps", bufs=2, space="PSUM"))
    pvs = ctx.enter_context(tc.tile_pool(name="pvs", bufs=2, space="PSUM"))

    wpool = ctx.enter_context(tc.tile_pool(name="wpool", bufs=2))
    stream = ctx.enter_context(tc.tile_pool(name="stream", bufs=3))
    ptp = ctx.enter_context(tc.tile_pool(name="ptp", bufs=4))
    rp = ctx.enter_context(tc.tile_pool(name="rp", bufs=6))
    op = ctx.enter_context(tc.tile_pool(name="op", bufs=3))

    def proj_in(name):
        # one DMA: [1024, 512] HBM slab -> [128, 8, 512] SBUF tile
        t = wpool.tile([128, 8, DH], BF16, tag="w", name="w")
        nc.sync.dma_start(t[:], io[name].rearrange("(k p) d -> p k d", k=8))
        return t

    def x_chunk(name, sc):
        t = stream.tile([128, 8, 512], BF16, tag="s", name="s")
        nc.sync.dma_start(
            t[:], io[name].rearrange("(k p) s -> p k s", k=8)[:, :, TS(sc, 512)])
        return t

    # ---------------- K projection (full) ----------------
    wk = proj_in("wkT")
    for sc in range(4):
        xs = x_chunk("kT", sc)
        for pt in range(NPT):
            psq = pp.tile([128, 512], F32, tag="pp", name="pp")
            for k in range(8):
                nc.tensor.matmul(
                    psq[:],
                    lhsT=wk[:, k, TS(pt, 128)],
                    rhs=xs[:, k, :],
                    start=(k == 0), stop=(k == 7), skip_group_check=True)
            nc.vector.tensor_copy(kT_sb[pt][:, TS(sc, 512)], psq[:])

    # ---------------- V projection (full) ----------------
    wv = proj_in("wvT")
    for st in range(16):
        vv = v_sb[st].rearrange("p (h e) -> p h e", e=65)
        nc.vector.memset(vv[:, :, 64:65], 1.0)
    for sc in range(4):
        xs = x_chunk("vT", sc)
        for j in range(4):
            st = sc * 4 + j
            psv = pp.tile([128, DH], F32, tag="pp", name="pp")
            for k in range(8):
                nc.tensor.matmul(
                    psv[:],
                    lhsT=xs[:, k, TS(j, 128)],
                    rhs=wv[:, k, :],
                    start=(k == 0), stop=(k == 7), skip_group_check=True)
            vv = v_sb[st].rearrange("p (h e) -> p h e", e=65)
            pv_view = psv.rearrange("p (h e) -> p h e", e=64)
            nc.vector.tensor_copy(vv[:, :, 0:64], pv_view[:, :, :])

    # ---------------- Q projection (per chunk) + attention ----------------
    wq = proj_in("wqT")
    patv = pat.rearrange("p (h w) -> p h w", h=2)

    for ch in range(NCH):
        # Q chunk ch: qT_sb[:, ch*512 : ...] for all pairs
        xs = x_chunk("qT", ch)
        for pt in range(NPT):
            psq = pp.tile([128, 512], F32, tag="pp", name="pp")
            for k in range(8):
                nc.tensor.matmul(
                    psq[:],
                    lhsT=wq[:, k, TS(pt, 128)],
                    rhs=xs[:, k, :],
                    start=(k == 0), stop=(k == 7), skip_group_check=True)
            # copy with per-partition Q bias on the scalar engine
            nc.scalar.activation(
                qT_sb[pt][:, TS(ch, 512)], psq[:], IDENT,
                bias=bqT[:, DS(pt, 1)])

        c0 = ch * 512
        for pt in range(NPT):
            pvt = [pvs.tile([65, 512], F32, tag="pv", name="pv") for _ in range(2)]
            ptts = [None] * NKJ

            def pv_step(kj):
                for h in (0, 1):
                    lh = pt * 2 + h
                    nc.tensor.matmul(
                        pvt[h][:],
                        lhsT=v_sb[kj][:, DS(lh * 65, 65)],
                        rhs=ptts[kj][:, DS(h * 512, 512)],
                        start=(kj == 0), stop=(kj == 15),
                        skip_group_check=True)

            for kj in range(16):
                kj0 = kj * 128
                sp = sps.tile([128, 1024], F32, tag="sp", name="sp")
                for h in (0, 1):
                    nc.tensor.matmul(
                        sp[:, DS(h * 512, 512)],
                        lhsT=kT_sb[pt][DS(h * 64, 64), TS(kj, 128)],
                        rhs=qT_sb[pt][DS(h * 64, 64), TS(ch, 512)],
                        start=True, stop=True,
                        tile_position=(h * 64, 0),
                        skip_group_check=True)
                ptt = ptp.tile([128, 1024], BF16, tag="ptt", name="ptt")
                nc.scalar.activation(ptt[:], sp[:], EXP, scale=0.125)
                ptts[kj] = ptt
                lo = max(kj0 - W, c0)
                hi = min(kj0 + 128 + W, c0 + 512)
                if lo < hi:
                    pa = lo - (kj0 - W)
                    pv3 = ptt.rearrange("p (h w) -> p h w", h=2)
                    nc.vector.tensor_mul(
                        pv3[:, :, DS(lo - c0, hi - lo)],
                        pv3[:, :, DS(lo - c0, hi - lo)],
                        patv[:, :, DS(pa, hi - lo)])
                if kj > 0:
                    pv_step(kj - 1)
            pv_step(15)

            for h in (0, 1):
                r1 = rp.tile([1, 512], F32, tag="r1", name="r1")
                nc.vector.reciprocal(r1[:], pvt[h][DS(64, 1), :])
                rb = rp.tile([64, 512], F32, tag="rb", name="rb")
                nc.gpsimd.partition_broadcast(rb[:], r1[:])
                nc.vector.tensor_mul(
                    a_sb[pt][DS(h * 64, 64), TS(ch, 512)],
                    pvt[h][DS(0, 64), :], rb[:])

        for j in range(4):
            st = ch * 4 + j
            ot = op.tile([128, M], BF16, tag="ot", name="ot")
            for mt in range(2):
                pso = pp.tile([128, 512], F32, tag="pp", name="pp")
                for pt in range(NPT):
                    nc.tensor.matmul(
                        pso[:],
                        lhsT=a_sb[pt][:, TS(st, 128)],
                        rhs=woT_sb[:, pt, TS(mt, 512)],
                        start=(pt == 0), stop=(pt == 3),
                        skip_group_check=True)
                nc.vector.tensor_copy(ot[:, TS(mt, 512)], pso[:])
            nc.sync.dma_start(io["out"][TS(st, 128), :], ot[:])


_CACHE = {}


def _build():
    if "nc" in _CACHE:
        return _CACHE["nc"]
    nc = bacc.Bacc("TRN2", target_bir_lowering=False, debug=False)
    io = {}
    for name, shape in (
        ("qT", [M, S]), ("kT", [M, S]), ("vT", [M, S]),
        ("wqT", [M, DH]), ("wkT", [M, DH]), ("wvT", [M, DH]),
        ("woT", [DH, M]),
    ):
        io[name] = nc.dram_tensor(name, shape, BF16, kind="ExternalInput").ap()
    io["pat"] = nc.dram_tensor("pat", [128, 320], BF16, kind="ExternalInput").ap()
    io["bqT"] = nc.dram_tensor("bqT", [128, NPT], F32, kind="ExternalInput").ap()
    io["out"] = nc.dram_tensor("out", [S, M], BF16, kind="ExternalOutput").ap()
    with tile.TileContext(nc) as tc:
        with ExitStack() as ctx:
            _emit(ctx, tc, io)
    nc.compile()
    _CACHE["nc"] = nc
    return nc


def _bias_pattern(local_bias):
    # multiplicative band pattern: exp(2*b[qi-kj+W]) inside the band, 1.0
    # outside; duplicated side by side for the two heads of a pair tile.
    p = np.arange(128)[:, None]
    f = np.arange(160)[None, :]
    idx = f - p  # rel + W
    valid = (idx >= 0) & (idx <= 2 * W)
    b = np.asarray(local_bias, np.float64)
    pat = np.where(valid, np.exp(2.0 * b[np.clip(idx, 0, 2 * W)]), 1.0)
    pat2 = np.concatenate([pat, pat], axis=1)
    return np.ascontiguousarray(pat2).astype(BF)


def kernel(query, key, value, wq, bq, wk, bk, wv, bv, wo, bo, local_bias):
    query = np.asarray(query, np.float32)
    key = np.asarray(key, np.float32)
    value = np.asarray(value, np.float32)
    wq, wk, wv, wo = (np.asarray(x, np.float32) for x in (wq, wk, wv, wo))
    bq, bk, bv, bo = (np.asarray(x, np.float32) for x in (bq, bk, bv, bo))
    pat = _bias_pattern(local_bias)

    nc = _build()
    in_maps = []
    for c in range(8):
        b, g = c // 2, c % 2
        sl = slice(g * DH, (g + 1) * DH)
        # bq shard laid out [128, NPT]: column pt = bias dims pt*128..+128
        bq_t = np.ascontiguousarray(
            bq[sl].reshape(NPT, 128).T).astype(np.float32)
        in_maps.append({
            "qT": np.ascontiguousarray(query[b].T).astype(BF),
            "kT": np.ascontiguousarray(key[b].T).astype(BF),
            "vT": np.ascontiguousarray(value[b].T).astype(BF),
            "wqT": np.ascontiguousarray(wq[sl, :].T).astype(BF),
            "wkT": np.ascontiguousarray(wk[sl, :].T).astype(BF),
            "wvT": np.ascontiguousarray(wv[sl, :].T).astype(BF),
            "woT": np.ascontiguousarray(wo[:, sl].T).astype(BF),
            "bqT": bq_t,
            "pat": pat,
        })
    res = run_bass_kernel_spmd(
        nc, in_maps, core_ids=list(range(8)),
        trace=bool(int(os.environ.get("KERNEL_TRACE", "0"))),
    )
    _CACHE["last_result"] = res
    outs = [np.asarray(r["out"], np.float32) for r in res.results]
    # host folds: V-bias (softmax rows sum to 1) and the output bias
    cvec = (wo @ bv + bo).astype(np.float32)
    out = np.stack([outs[2 * b] + outs[2 * b + 1] + cvec for b in range(4)])
    return out.astype(np.float32)


# revision 7
# speedup vs baseline: 1.1206x; 1.0568x over previous
"""LocalityEnhancedAttention Trainium2 kernel (8 NeuronCores, SPMD).

Sharding: core c handles batch b = c//2 and head-group g = c%2
(8 of 16 heads). Each core computes its partial output projection
(attn_heads @ wo_shard.T); host sums the two head-group partials per
batch and adds bo + wo@bv (the V-bias folds out of the device entirely
because softmax rows sum to one; the K-bias shifts every logit in a row
equally so it cancels in softmax; the Q-bias is added during the Q
projection's PSUM->SBUF copy as a per-partition ACT bias).

Device dataflow per core (S=2048, M=1024, local head-dims DH=512):
  - host pre-transposes inputs:  qT/kT/vT = x[b].T  [M, S]
  - projections: K^T, V, then Q^T chunk-by-chunk; attention for query
    chunk 0 is emitted right after Q chunk 0 so the Q tail overlaps.
  - scores^T[kj, qi] = K^T.T @ Q^T per head, head pairs packed into
    PE row-halves, exp via ACT (scale=1/8 folded), banded local bias
    applied multiplicatively on the bf16 probs (DVE), P^T in SBUF.
  - PV: A^T_aug[65, qi] += V_aug[kj].T @ P^T[kj] in PSUM; row 64 =
    softmax denominators; PV lags scores by one kj tile so the tensor
    queue never blocks on the exp of the tile it just produced.
  - normalize: reciprocal_approx_fast + gpsimd partition_broadcast +
    DVE mul -> A_norm^T in SBUF (bf16).
  - WO: out_partial[s, m] = sum_d A_norm^T.T @ woT  (bf16 out).
"""

import os
import sys
from contextlib import ExitStack

import numpy as np

sys.path.insert(0, "/opt/trn_rl_repo")

import ml_dtypes

BF = ml_dtypes.bfloat16

import concourse.bass as bass
import concourse.mybir as mybir
import concourse.tile as tile
from concourse import bacc
from concourse.bass_utils import run_bass_kernel_spmd

F32 = mybir.dt.float32
BF16 = mybir.dt.bfloat16
EXP = mybir.ActivationFunctionType.Exp
IDENT = mybir.ActivationFunctionType.Identity
TS, DS = bass.ts, bass.ds

S = 2048
M = 1024
DH = 512        # head dims per core (8 heads x 64)
DK = 64
W = 16
NPT = 4         # head pairs per core
NCH = 4         # qi chunks of 512
NKJ = 16        # kj tiles of 128


def _emit(ctx, tc, io):
    nc = tc.nc

    const = ctx.enter_context(tc.tile_pool(name="const", bufs=1))
    qkvp = ctx.enter_context(tc.tile_pool(name="qkv", bufs=1))
    ap_ = ctx.enter_context(tc.tile_pool(name="anorm", bufs=1))
    wop = ctx.enter_context(tc.tile_pool(name="wop", bufs=1))

    pat = const.tile([128, 320], BF16, tag="pat", name="pat")
    nc.sync.dma_start(pat[:], io["pat"])
    bqT = const.tile([128, NPT], F32, tag="bqT", name="bqT")
    nc.sync.dma_start(bqT[:], io["bqT"])

    qT_sb = [qkvp.tile([128, S], BF16, tag=f"q{i}", name=f"q{i}") for i in range(NPT)]
    kT_sb = [qkvp.tile([128, S], BF16, tag=f"k{i}", name=f"k{i}") for i in range(NPT)]
    v_sb = [qkvp.tile([128, 8 * 65], BF16, tag=f"v{i}", name=f"v{i}") for i in range(16)]
    a_sb = [ap_.tile([128, S], BF16, tag=f"a{i}", name=f"a{i}") for i in range(NPT)]
    woT_sb = wop.tile([128, NPT, M], BF16, tag="wo", name="wo")
    nc.sync.dma_start(
        woT_sb[:], io["woT"].rearrange("(t p) m -> p t m", t=NPT))

    # shared PSUM pool: projection accumulators AND the WO accumulators
    pp = ctx.enter_context(tc.tile_pool(name="pp", bufs=2, space="PSUM"))
    # attention PSUM pools
    sps = ctx.enter_context(tc.tile_pool(name="sps", bufs=2, space="PSUM"))
    pvs = ctx.enter_context(tc.tile_pool(name="pvs", bufs=2, space="PSUM"))

    wpool = ctx.enter_context(tc.tile_pool(name="wpool", bufs=2))
    stream = ctx.enter_context(tc.tile_pool(name="stream", bufs=3))
    ptp = ctx.enter_context(tc.tile_pool(name="ptp", bufs=4))
    rp = ctx.enter_context(tc.tile_pool(name="rp", bufs=6))
    op = ctx.enter_context(tc.tile_pool(name="op", bufs=3))

    def proj_in(name):
        # one DMA: [1024, 512] HBM slab -> [128, 8, 512] SBUF tile
        t = wpool.tile([128, 8, DH], BF16, tag="w", name="w")
        nc.sync.dma_start(t[:], io[name].rearrange("(k p) d -> p k d", k=8))
        return t

    def x_chunk(name, sc):
        t = stream.tile([128, 8, 512], BF16, tag="s", name="s")
        nc.sync.dma_start(
            t[:], io[name].rearrange("(k p) s -> p k s", k=8)[:, :, TS(sc, 512)])
        return t

    # ---------------- K projection (full) ----------------
    wk = proj_in("wkT")
    for sc in range(4):
        xs = x_chunk("kT", sc)
        for pt in range(NPT):
            psq = pp.tile([128, 512], F32, tag="pp", name="pp")
            for k in range(8):
                nc.tensor.matmul(
                    psq[:],
                    lhsT=wk[:, k, TS(pt, 128)],
                    rhs=xs[:, k, :],
                    start=(k == 0), stop=(k == 7), skip_group_check=True)
            nc.vector.tensor_copy(kT_sb[pt][:, TS(sc, 512)], psq[:])

    # ---------------- V projection (full) ----------------
    wv = proj_in("wvT")
    for st in range(16):
        vv = v_sb[st].rearrange("p (h e) -> p h e", e=65)
        nc.vector.memset(vv[:, :, 64:65], 1.0)
    for sc in range(4):
        xs = x_chunk("vT", sc)
        for j in range(4):
            st = sc * 4 + j
            psv = pp.tile([128, DH], F32, tag="pp", name="pp")
            for k in range(8):
                nc.tensor.matmul(
                    psv[:],
                    lhsT=xs[:, k, TS(j, 128)],
                    rhs=wv[:, k, :],
                    start=(k == 0), stop=(k == 7), skip_group_check=True)
            vv = v_sb[st].rearrange("p (h e) -> p h e", e=65)
            pv_view = psv.rearrange("p (h e) -> p h e", e=64)
            nc.vector.tensor_copy(vv[:, :, 0:64], pv_view[:, :, :])

    # ---------------- Q projection (per chunk) + attention ----------------
    wq = proj_in("wqT")
    patv = pat.rearrange("p (h w) -> p h w", h=2)

    def q_proj_chunk(ch, pt, xs):
        psq = pp.tile([128, 512], F32, tag="pp", name="pp")
        for k in range(8):
            nc.tensor.matmul(
                psq[:],
                lhsT=wq[:, k, TS(pt, 128)],
                rhs=xs[:, k, :],
                start=(k == 0), stop=(k == 7), skip_group_check=True)
        # copy + per-partition Q bias on DVE
        nc.vector.tensor_scalar_add(
            qT_sb[pt][:, TS(ch, 512)], psq[:], bqT[:, DS(pt, 1)])

    # Q chunk 0 up front; chunk ch+1 is emitted inside chunk ch's
    # attention (in the normalize shadow at pair boundaries).
    xs_q = x_chunk("qT", 0)
    for pt in range(NPT):
        q_proj_chunk(0, pt, xs_q)

    for ch in range(NCH):
        if ch + 1 < NCH:
            xs_q = x_chunk("qT", ch + 1)
        c0 = ch * 512
        for pt in range(NPT):
            pvt = [pvs.tile([65, 512], F32, tag="pv", name="pv") for _ in range(2)]
            ptts = [None] * NKJ

            def pv_step(kj):
                for h in (0, 1):
                    lh = pt * 2 + h
                    nc.tensor.matmul(
                        pvt[h][:],
                        lhsT=v_sb[kj][:, DS(lh * 65, 65)],
                        rhs=ptts[kj][:, DS(h * 512, 512)],
                        start=(kj == 0), stop=(kj == 15),
                        skip_group_check=True)

            for kj in range(16):
                kj0 = kj * 128
                sp = sps.tile([128, 1024], F32, tag="sp", name="sp")
                for h in (0, 1):
                    nc.tensor.matmul(
                        sp[:, DS(h * 512, 512)],
                        lhsT=kT_sb[pt][DS(h * 64, 64), TS(kj, 128)],
                        rhs=qT_sb[pt][DS(h * 64, 64), TS(ch, 512)],
                        start=True, stop=True,
                        tile_position=(h * 64, 0),
                        skip_group_check=True)
                ptt = ptp.tile([128, 1024], BF16, tag="ptt", name="ptt")
                nc.scalar.activation(ptt[:], sp[:], EXP, scale=0.125)
                ptts[kj] = ptt
                lo = max(kj0 - W, c0)
                hi = min(kj0 + 128 + W, c0 + 512)
                if lo < hi:
                    pa = lo - (kj0 - W)
                    pv3 = ptt.rearrange("p (h w) -> p h w", h=2)
                    nc.vector.tensor_mul(
                        pv3[:, :, DS(lo - c0, hi - lo)],
                        pv3[:, :, DS(lo - c0, hi - lo)],
                        patv[:, :, DS(pa, hi - lo)])
                if kj > 0:
                    pv_step(kj - 1)
            pv_step(15)

            # independent tensor work to cover the normalize latency
            if ch + 1 < NCH:
                q_proj_chunk(ch + 1, pt, xs_q)

            for h in (0, 1):
                r1 = rp.tile([1, 512], F32, tag="r1", name="r1")
                nc.vector.reciprocal(r1[:], pvt[h][DS(64, 1), :])
                rb = rp.tile([64, 512], F32, tag="rb", name="rb")
                nc.gpsimd.partition_broadcast(rb[:], r1[:])
                nc.vector.tensor_mul(
                    a_sb[pt][DS(h * 64, 64), TS(ch, 512)],
                    pvt[h][DS(0, 64), :], rb[:])

        for j in range(4):
            st = ch * 4 + j
            ot = op.tile([128, M], BF16, tag="ot", name="ot")
            for mt in range(2):
                pso = pp.tile([128, 512], F32, tag="pp", name="pp")
                for pt in range(NPT):
                    nc.tensor.matmul(
                        pso[:],
                        lhsT=a_sb[pt][:, TS(st, 128)],
                        rhs=woT_sb[:, pt, TS(mt, 512)],
                        start=(pt == 0), stop=(pt == 3),
                        skip_group_check=True)
                nc.vector.tensor_copy(ot[:, TS(mt, 512)], pso[:])
            nc.sync.dma_start(io["out"][TS(st, 128), :], ot[:])


_CACHE = {}


def _build():
    if "nc" in _CACHE:
        return _CACHE["nc"]
    nc = bacc.Bacc("TRN2", target_bir_lowering=False, debug=False)
    io = {}
    for name, shape in (
        ("qT", [M, S]), ("kT", [M, S]), ("vT", [M, S]),
        ("wqT", [M, DH]), ("wkT", [M, DH]), ("wvT", [M, DH]),
        ("woT", [DH, M]),
    ):
        io[name] = nc.dram_tensor(name, shape, BF16, kind="ExternalInput").ap()
    io["pat"] = nc.dram_tensor("pat", [128, 320], BF16, kind="ExternalInput").ap()
    io["bqT"] = nc.dram_tensor("bqT", [128, NPT], F32, kind="ExternalInput").ap()
    io["out"] = nc.dram_tensor("out", [S, M], BF16, kind="ExternalOutput").ap()
    with tile.TileContext(nc) as tc:
        with ExitStack() as ctx:
            _emit(ctx, tc, io)
    nc.compile()
    _CACHE["nc"] = nc
    return nc


def _bias_pattern(local_bias):
    # multiplicative band pattern: exp(2*b[qi-kj+W]) inside the band, 1.0
    # outside; duplicated side by side for the two heads of a pair tile.
    p = np.arange(128)[:, None]
    f = np.arange(160)[None, :]
    idx = f - p  # rel + W
    valid = (idx >= 0) & (idx <= 2 * W)
    b = np.asarray(local_bias, np.float64)
    pat = np.where(valid, np.exp(2.0 * b[np.clip(idx, 0, 2 * W)]), 1.0)
    pat2 = np.concatenate([pat, pat], axis=1)
    return np.ascontiguousarray(pat2).astype(BF)


def kernel(query, key, value, wq, bq, wk, bk, wv, bv, wo, bo, local_bias):
    query = np.asarray(query, np.float32)
    key = np.asarray(key, np.float32)
    value = np.asarray(value, np.float32)
    wq, wk, wv, wo = (np.asarray(x, np.float32) for x in (wq, wk, wv, wo))
    bq, bk, bv, bo = (np.asarray(x, np.float32) for x in (bq, bk, bv, bo))
    pat = _bias_pattern(local_bias)

    nc = _build()
    in_maps = []
    for c in range(8):
        b, g = c // 2, c % 2
        sl = slice(g * DH, (g + 1) * DH)
        # bq shard laid out [128, NPT]: column pt = bias dims pt*128..+128
        bq_t = np.ascontiguousarray(
            bq[sl].reshape(NPT, 128).T).astype(np.float32)
        in_maps.append({
            "qT": np.ascontiguousarray(query[b].T).astype(BF),
            "kT": np.ascontiguousarray(key[b].T).astype(BF),
            "vT": np.ascontiguousarray(value[b].T).astype(BF),
            "wqT": np.ascontiguousarray(wq[sl, :].T).astype(BF),
            "wkT": np.ascontiguousarray(wk[sl, :].T).astype(BF),
            "wvT": np.ascontiguousarray(wv[sl, :].T).astype(BF),
            "woT": np.ascontiguousarray(wo[:, sl].T).astype(BF),
            "bqT": bq_t,
            "pat": pat,
        })
    res = run_bass_kernel_spmd(
        nc, in_maps, core_ids=list(range(8)),
        trace=bool(int(os.environ.get("KERNEL_TRACE", "0"))),
    )
    _CACHE["last_result"] = res
    outs = [np.asarray(r["out"], np.float32) for r in res.results]
    # host folds: V-bias (softmax rows sum to 1) and the output bias
    cvec = (wo @ bv + bo).astype(np.float32)
    out = np.stack([outs[2 * b] + outs[2 * b + 1] + cvec for b in range(4)])
    return out.astype(np.float32)
